# revision 3
# baseline (speedup 1.0000x reference)
"""Trainium2 Bass kernel for nn_DescriptorMatcher (retrieval_knn, 8 cores).

Nearest-neighbour matching: for each of 8192 desc1 rows find the closest
(L2) of 8192 desc2 rows.  Sharding: desc1 rows split across 8 cores
(1024 rows each); desc2 replicated.

Per-core device pipeline (desc1 rows on partitions, desc2 index j on the
free dim; score s_neg[i,j] = -(|d2_j|^2 - 2<d1_i,d2_j>) so argmin dist ==
argmax s_neg):
  - PE:  psum = (-2*d1_slab) @ d2.T        (fp32 matmuls, 512-wide)
  - DVE: tensor_tensor sub: s_neg = (-q_bcast) - psum   (evict+bias fused)
  - DVE: max (top-8) -> m8;  max_index -> first j with s_neg == max
         (exact first-occurrence tie semantics, matching jnp.argmin)
Host: dist = sqrt(clip(|d1_i|^2 - gmax, 0)); idxs assembled as int32.
"""

import numpy as np

B1, B2, D = 8192, 8192, 128
NCORES = 8
M = B1 // NCORES          # 1024 desc1 rows per core
ITILES = M // 128         # 8 i-tiles per core
FD = 2048                 # columns per evict TT (4 PSUM banks)
NG = B2 // FD             # evict groups per i-tile
MMW = 512                 # fp32 matmul moving-operand width
NT = FD // MMW            # matmuls per group

_CACHE = {}


def _build_nc():
    import concourse.bacc as bacc
    import concourse.mybir as mybir
    from concourse.tile import TileContext

    f32 = mybir.dt.float32
    nc = bacc.Bacc(
        "TRN2", target_bir_lowering=False, debug=False, num_devices=NCORES
    )
    d1t = nc.declare_dram_parameter("d1t", [D, M], f32, isOutput=False)
    d2t = nc.declare_dram_parameter("d2t", [D, B2], f32, isOutput=False)
    qnb = nc.declare_dram_parameter("qnb", [128, B2], f32, isOutput=False)
    o_v = nc.declare_dram_parameter("o_v", [ITILES, 128], f32, isOutput=True)
    o_i = nc.declare_dram_parameter("o_i", [ITILES, 128], f32, isOutput=True)

    with TileContext(nc) as tc:
        with (
            tc.tile_pool(name="const", bufs=1) as cpool,
            tc.tile_pool(name="sneg", bufs=2) as spool,
            tc.tile_pool(name="ps", bufs=2, space="PSUM") as ppool,
            tc.tile_pool(name="small", bufs=2) as mpool,
        ):
            d1s = cpool.tile([D, M], f32)
            d2s = cpool.tile([D, B2], f32)
            qns = cpool.tile([128, B2], f32)
            nc.sync.dma_start(out=d1s[:, :], in_=d1t[:, :])
            nc.sync.dma_start(out=d2s[:, :], in_=d2t[:, :])
            nc.sync.dma_start(out=qns[:, :], in_=qnb[:, :])

            for it in range(ITILES):
                sneg = spool.tile([128, B2], f32, tag="sneg")
                lhs = d1s[:, it * 128 : (it + 1) * 128]
                for g in range(NG):
                    pg = ppool.tile([128, FD], f32, tag="pg")
                    for t in range(NT):
                        nc.tensor.matmul(
                            pg[:, t * MMW : (t + 1) * MMW],
                            lhsT=lhs,
                            rhs=d2s[
                                :, g * FD + t * MMW : g * FD + (t + 1) * MMW
                            ],
                            start=True,
                            stop=True,
                        )
                    nc.vector.tensor_tensor(
                        out=sneg[:, g * FD : (g + 1) * FD],
                        in0=qns[:, g * FD : (g + 1) * FD],
                        in1=pg[:, :],
                        op=mybir.AluOpType.subtract,
                    )
                m8 = mpool.tile([128, 8], f32, tag="m8")
                nc.vector.max(out=m8[:, :], in_=sneg[:, :])
                idx8 = mpool.tile([128, 8], mybir.dt.uint32, tag="idx8")
                nc.vector.max_index(
                    out=idx8[:, :], in_max=m8[:, :], in_values=sneg[:, :]
                )
                idxf = mpool.tile([128, 1], f32, tag="idxf")
                nc.vector.tensor_copy(idxf[:, :], idx8[:, 0:1])
                nc.sync.dma_start(out=o_v[it, :], in_=m8[:, 0])
                nc.sync.dma_start(out=o_i[it, :], in_=idxf[:, 0])
    nc.compile()
    return nc


def _get_nc():
    if "nc" not in _CACHE:
        _CACHE["nc"] = _build_nc()
    return _CACHE["nc"]


def _prep_inputs(desc1, desc2):
    d1 = np.asarray(desc1, dtype=np.float32)
    d2 = np.asarray(desc2, dtype=np.float32)
    q = (d2.astype(np.float64) ** 2).sum(axis=1).astype(np.float32)
    d2t = np.ascontiguousarray(d2.T)
    qnb = np.ascontiguousarray(np.broadcast_to(-q[None, :], (128, B2)))
    in_maps = []
    for c in range(NCORES):
        slab = d1[c * M : (c + 1) * M]
        d1t = np.ascontiguousarray((-2.0 * slab).T)
        in_maps.append({"d1t": d1t, "d2t": d2t, "qnb": qnb})
    return in_maps


def _install_ntff_shim():
    """Register antenv.axon_hooks (missing from this image) so
    run_bass_kernel_spmd(trace=True) can drive NTFF profiling via
    libaxon_pjrt.so's C ABI. Also neuter the cloud artifact upload."""
    import contextlib
    import ctypes
    import sys
    import types

    import concourse.bass_utils as bu

    bu.upload_artifacts = lambda tmpdir: "local://" + str(tmpdir)
    if "antenv.axon_hooks" in sys.modules:
        return
    so_path = "/opt/axon/libaxon_pjrt.so"
    lib = ctypes.CDLL(so_path)
    if not hasattr(lib, "axon_start_nrt_profile"):
        hook = None
    else:
        lib.axon_start_nrt_profile.argtypes = [
            ctypes.POINTER(ctypes.c_int64),
            ctypes.c_size_t,
        ]
        lib.axon_start_nrt_profile.restype = ctypes.c_int64
        lib.axon_stop_nrt_profile.argtypes = [ctypes.c_char_p]
        lib.axon_stop_nrt_profile.restype = ctypes.c_int64

        @contextlib.contextmanager
        def hook(output_dir, device_ids):
            import jax

            jax.devices()
            if device_ids:
                ids = (ctypes.c_int64 * len(device_ids))(*device_ids)
                rc = lib.axon_start_nrt_profile(ids, len(device_ids))
            else:
                rc = lib.axon_start_nrt_profile(None, 0)
            if rc != 0:
                raise RuntimeError(f"axon_start_nrt_profile rc={rc}")
            try:
                yield
            finally:
                n = lib.axon_stop_nrt_profile(str(output_dir).encode())
                print(f"profile: {n} file(s) written to {output_dir}")

    mod = types.ModuleType("antenv.axon_hooks")
    mod.get_axon_ntff_profile_hook = lambda: hook
    mod.set_axon_ntff_profile_hook = lambda h: None
    import antenv

    sys.modules["antenv.axon_hooks"] = mod
    antenv.axon_hooks = mod


def kernel(desc1, desc2, _trace=False, _tmpdir=None):
    from concourse.bass_utils import run_bass_kernel_spmd

    if _trace:
        _install_ntff_shim()

    d1 = np.asarray(desc1, dtype=np.float32)
    nc = _get_nc()
    in_maps = _prep_inputs(d1, desc2)
    res = run_bass_kernel_spmd(
        nc,
        in_maps,
        list(range(NCORES)),
        trace=_trace,
        tmpdir=_tmpdir,
    )
    _CACHE["last_results"] = res

    gmax = np.concatenate(
        [res.results[c]["o_v"].reshape(-1) for c in range(NCORES)]
    )
    idxf = np.concatenate(
        [res.results[c]["o_i"].reshape(-1) for c in range(NCORES)]
    )
    d1sq = (d1.astype(np.float64) ** 2).sum(axis=1).astype(np.float32)
    dist2 = np.maximum(d1sq - gmax, 0.0).astype(np.float32)
    match_dists = np.sqrt(dist2).astype(np.float32)[:, None]
    idxs2 = idxf.astype(np.int32)
    idxs1 = np.arange(B1, dtype=np.int32)
    matches_idxs = np.stack([idxs1, idxs2], axis=1)
    return match_dists, matches_idxs


# revision 4
# speedup vs baseline: 1.0107x; 1.0107x over previous
"""Trainium2 Bass kernel for nn_DescriptorMatcher (retrieval_knn, 8 cores).

Nearest-neighbour matching: for each of 8192 desc1 rows find the closest
(L2) of 8192 desc2 rows.  Sharding: desc1 rows split across 8 cores
(1024 rows each); desc2 replicated.

Per-core device pipeline (desc1 rows on partitions, desc2 index j on the
free dim; score s_neg[i,j] = -(|d2_j|^2 - 2<d1_i,d2_j>) so argmin dist ==
argmax s_neg):
  - PE:  psum = (-2*d1_slab) @ d2.T        (fp32 matmuls, 512-wide)
  - DVE: tensor_tensor sub: s_neg = (-q_bcast) - psum   (evict+bias fused)
  - DVE: max (top-8) -> m8;  max_index -> first j with s_neg == max
         (exact first-occurrence tie semantics, matching jnp.argmin)
Host: dist = sqrt(clip(|d1_i|^2 - gmax, 0)); idxs assembled as int32.
"""

import numpy as np

B1, B2, D = 8192, 8192, 128
NCORES = 8
M = B1 // NCORES          # 1024 desc1 rows per core
ITILES = M // 128         # 8 i-tiles per core
FD = 2048                 # columns per evict TT (4 PSUM banks)
NG = B2 // FD             # evict groups per i-tile
MMW = 512                 # fp32 matmul moving-operand width
NT = FD // MMW            # matmuls per group

_CACHE = {}


def _build_nc():
    import concourse.bacc as bacc
    import concourse.mybir as mybir
    from concourse.tile import TileContext

    f32 = mybir.dt.float32
    nc = bacc.Bacc(
        "TRN2", target_bir_lowering=False, debug=False, num_devices=NCORES
    )
    d1t = nc.declare_dram_parameter("d1t", [D, M], f32, isOutput=False)
    d2t = nc.declare_dram_parameter("d2t", [D, B2], f32, isOutput=False)
    qnb = nc.declare_dram_parameter("qnb", [128, B2], f32, isOutput=False)
    o_v = nc.declare_dram_parameter("o_v", [ITILES, 128], f32, isOutput=True)
    o_i = nc.declare_dram_parameter("o_i", [ITILES, 128], f32, isOutput=True)

    with TileContext(nc) as tc:
        with (
            tc.tile_pool(name="const", bufs=1) as cpool,
            tc.tile_pool(name="sneg", bufs=2) as spool,
            tc.tile_pool(name="ps", bufs=2, space="PSUM") as ppool,
            tc.tile_pool(name="small", bufs=2) as mpool,
        ):
            d1s = cpool.tile([D, M], f32)
            nc.sync.dma_start(out=d1s[:, :], in_=d1t[:, :])
            # Chunked loads: group g's matmuls only wait for their own
            # 2048-column slab instead of the full 4MB transfer.
            d2g = []
            qng = []
            for g in range(NG):
                dt_ = cpool.tile([D, FD], f32, tag=f"d2g{g}")
                qt_ = cpool.tile([128, FD], f32, tag=f"qng{g}")
                nc.sync.dma_start(
                    out=dt_[:, :], in_=d2t[:, g * FD : (g + 1) * FD]
                )
                nc.sync.dma_start(
                    out=qt_[:, :], in_=qnb[:, g * FD : (g + 1) * FD]
                )
                d2g.append(dt_)
                qng.append(qt_)

            for it in range(ITILES):
                sneg = spool.tile([128, B2], f32, tag="sneg")
                lhs = d1s[:, it * 128 : (it + 1) * 128]
                for g in range(NG):
                    pg = ppool.tile([128, FD], f32, tag="pg")
                    for t in range(NT):
                        nc.tensor.matmul(
                            pg[:, t * MMW : (t + 1) * MMW],
                            lhsT=lhs,
                            rhs=d2g[g][:, t * MMW : (t + 1) * MMW],
                            start=True,
                            stop=True,
                        )
                    nc.vector.tensor_tensor(
                        out=sneg[:, g * FD : (g + 1) * FD],
                        in0=qng[g][:, :],
                        in1=pg[:, :],
                        op=mybir.AluOpType.subtract,
                    )
                m8 = mpool.tile([128, 8], f32, tag="m8")
                nc.vector.max(out=m8[:, :], in_=sneg[:, :])
                idx8 = mpool.tile([128, 8], mybir.dt.uint32, tag="idx8")
                nc.vector.max_index(
                    out=idx8[:, :], in_max=m8[:, :], in_values=sneg[:, :]
                )
                idxf = mpool.tile([128, 1], f32, tag="idxf")
                nc.vector.tensor_copy(idxf[:, :], idx8[:, 0:1])
                nc.sync.dma_start(out=o_v[it, :], in_=m8[:, 0])
                nc.sync.dma_start(out=o_i[it, :], in_=idxf[:, 0])
    nc.compile()
    return nc


def _get_nc():
    if "nc" not in _CACHE:
        _CACHE["nc"] = _build_nc()
    return _CACHE["nc"]


def _prep_inputs(desc1, desc2):
    d1 = np.asarray(desc1, dtype=np.float32)
    d2 = np.asarray(desc2, dtype=np.float32)
    q = (d2.astype(np.float64) ** 2).sum(axis=1).astype(np.float32)
    d2t = np.ascontiguousarray(d2.T)
    qnb = np.ascontiguousarray(np.broadcast_to(-q[None, :], (128, B2)))
    in_maps = []
    for c in range(NCORES):
        slab = d1[c * M : (c + 1) * M]
        d1t = np.ascontiguousarray((-2.0 * slab).T)
        in_maps.append({"d1t": d1t, "d2t": d2t, "qnb": qnb})
    return in_maps


def _install_ntff_shim():
    """Register antenv.axon_hooks (missing from this image) so
    run_bass_kernel_spmd(trace=True) can drive NTFF profiling via
    libaxon_pjrt.so's C ABI. Also neuter the cloud artifact upload."""
    import contextlib
    import ctypes
    import sys
    import types

    import concourse.bass_utils as bu

    bu.upload_artifacts = lambda tmpdir: "local://" + str(tmpdir)
    if "antenv.axon_hooks" in sys.modules:
        return
    so_path = "/opt/axon/libaxon_pjrt.so"
    lib = ctypes.CDLL(so_path)
    if not hasattr(lib, "axon_start_nrt_profile"):
        hook = None
    else:
        lib.axon_start_nrt_profile.argtypes = [
            ctypes.POINTER(ctypes.c_int64),
            ctypes.c_size_t,
        ]
        lib.axon_start_nrt_profile.restype = ctypes.c_int64
        lib.axon_stop_nrt_profile.argtypes = [ctypes.c_char_p]
        lib.axon_stop_nrt_profile.restype = ctypes.c_int64

        @contextlib.contextmanager
        def hook(output_dir, device_ids):
            import jax

            jax.devices()
            if device_ids:
                ids = (ctypes.c_int64 * len(device_ids))(*device_ids)
                rc = lib.axon_start_nrt_profile(ids, len(device_ids))
            else:
                rc = lib.axon_start_nrt_profile(None, 0)
            if rc != 0:
                raise RuntimeError(f"axon_start_nrt_profile rc={rc}")
            try:
                yield
            finally:
                n = lib.axon_stop_nrt_profile(str(output_dir).encode())
                print(f"profile: {n} file(s) written to {output_dir}")

    mod = types.ModuleType("antenv.axon_hooks")
    mod.get_axon_ntff_profile_hook = lambda: hook
    mod.set_axon_ntff_profile_hook = lambda h: None
    import antenv

    sys.modules["antenv.axon_hooks"] = mod
    antenv.axon_hooks = mod


def kernel(desc1, desc2, _trace=False, _tmpdir=None):
    from concourse.bass_utils import run_bass_kernel_spmd

    if _trace:
        _install_ntff_shim()

    d1 = np.asarray(desc1, dtype=np.float32)
    nc = _get_nc()
    in_maps = _prep_inputs(d1, desc2)
    res = run_bass_kernel_spmd(
        nc,
        in_maps,
        list(range(NCORES)),
        trace=_trace,
        tmpdir=_tmpdir,
    )
    _CACHE["last_results"] = res

    gmax = np.concatenate(
        [res.results[c]["o_v"].reshape(-1) for c in range(NCORES)]
    )
    idxf = np.concatenate(
        [res.results[c]["o_i"].reshape(-1) for c in range(NCORES)]
    )
    d1sq = (d1.astype(np.float64) ** 2).sum(axis=1).astype(np.float32)
    dist2 = np.maximum(d1sq - gmax, 0.0).astype(np.float32)
    match_dists = np.sqrt(dist2).astype(np.float32)[:, None]
    idxs2 = idxf.astype(np.int32)
    idxs1 = np.arange(B1, dtype=np.int32)
    matches_idxs = np.stack([idxs1, idxs2], axis=1)
    return match_dists, matches_idxs


# revision 5
# speedup vs baseline: 1.1771x; 1.1647x over previous
"""Trainium2 Bass kernel for nn_DescriptorMatcher (retrieval_knn, 8 cores).

Nearest-neighbour matching: for each of 8192 desc1 rows find the closest
(L2) of 8192 desc2 rows.  Sharding: desc1 rows split across 8 cores
(1024 rows each); desc2 replicated.

Per-core device pipeline (desc1 rows on partitions, desc2 index j on the
free dim; score s_neg[i,j] = -(|d2_j|^2 - 2<d1_i,d2_j>) so argmin dist ==
argmax s_neg):
  - PE:  psum = (-2*d1_slab) @ d2.T        (fp32 matmuls, 512-wide)
  - DVE: tensor_tensor sub: s_neg = (-q_bcast) - psum   (evict+bias fused)
  - DVE: max (top-8) -> m8;  max_index -> first j with s_neg == max
         (exact first-occurrence tie semantics, matching jnp.argmin)
Host: dist = sqrt(clip(|d1_i|^2 - gmax, 0)); idxs assembled as int32.
"""

import numpy as np

B1, B2, D = 8192, 8192, 128
NCORES = 8
M = B1 // NCORES          # 1024 desc1 rows per core
ITILES = M // 128         # 8 i-tiles per core
FD = 2048                 # columns per evict TT (4 PSUM banks)
NG = B2 // FD             # evict groups per i-tile
MMW = 512                 # fp32 matmul moving-operand width
NT = FD // MMW            # matmuls per group

_CACHE = {}


def _build_nc():
    import concourse.bacc as bacc
    import concourse.mybir as mybir
    from concourse.tile import TileContext

    f32 = mybir.dt.float32
    nc = bacc.Bacc(
        "TRN2", target_bir_lowering=False, debug=False, num_devices=NCORES
    )
    d1t = nc.declare_dram_parameter("d1t", [D, M], f32, isOutput=False)
    d2t = nc.declare_dram_parameter("d2t", [D, B2], f32, isOutput=False)
    qnb = nc.declare_dram_parameter("qnb", [128, B2], f32, isOutput=False)
    o_v = nc.declare_dram_parameter("o_v", [ITILES, 128], f32, isOutput=True)
    o_i = nc.declare_dram_parameter("o_i", [ITILES, 128], f32, isOutput=True)

    with TileContext(nc) as tc:
        with (
            tc.tile_pool(name="const", bufs=1) as cpool,
            tc.tile_pool(name="sneg", bufs=2) as spool,
            tc.tile_pool(name="ps", bufs=2, space="PSUM") as ppool,
            tc.tile_pool(name="small", bufs=2) as mpool,
        ):
            d1s = cpool.tile([D, M], f32)
            nc.sync.dma_start(out=d1s[:, :], in_=d1t[:, :])
            # Chunked loads: group g's matmuls only wait for their own
            # 2048-column slab instead of the full 4MB transfer.
            d2g = []
            qng = []
            for g in range(NG):
                dt_ = cpool.tile([D, FD], f32, tag=f"d2g{g}")
                qt_ = cpool.tile([128, FD], f32, tag=f"qng{g}")
                nc.sync.dma_start(
                    out=dt_[:, :], in_=d2t[:, g * FD : (g + 1) * FD]
                )
                nc.sync.dma_start(
                    out=qt_[:, :], in_=qnb[:, g * FD : (g + 1) * FD]
                )
                d2g.append(dt_)
                qng.append(qt_)

            ones8 = cpool.tile([128, 8], f32)
            nc.vector.memset(ones8[:, :], 1.0)

            for it in range(ITILES):
                sneg = spool.tile([128, B2], f32, tag="sneg")
                m4 = mpool.tile([128, NG], f32, tag="m4")
                lhs = d1s[:, it * 128 : (it + 1) * 128]
                for g in range(NG):
                    pg = ppool.tile([128, FD], f32, tag="pg")
                    for t in range(NT):
                        nc.tensor.matmul(
                            pg[:, t * MMW : (t + 1) * MMW],
                            lhsT=lhs,
                            rhs=d2g[g][:, t * MMW : (t + 1) * MMW],
                            start=True,
                            stop=True,
                        )
                    sl = slice(g * FD, (g + 1) * FD)
                    nc.vector.tensor_tensor(
                        out=sneg[:, sl],
                        in0=qng[g][:, :],
                        in1=pg[:, :],
                        op=mybir.AluOpType.subtract,
                    )
                    # per-group max at 2x mode (single-src ts-reduce);
                    # elementwise out is an exact *1.0 in-place rewrite
                    nc.vector.tensor_scalar(
                        sneg[:, sl],
                        sneg[:, sl],
                        1.0,
                        None,
                        op0=mybir.AluOpType.mult,
                        op1=mybir.AluOpType.max,
                        accum_out=m4[:, g : g + 1],
                    )
                gmax = mpool.tile([128, 1], f32, tag="gmax")
                nc.vector.tensor_reduce(
                    out=gmax[:, :], in_=m4[:, :], axis=mybir.AxisListType.X,
                    op=mybir.AluOpType.max,
                )
                gmax8 = mpool.tile([128, 8], f32, tag="gmax8")
                nc.vector.tensor_scalar(
                    gmax8[:, :], ones8[:, :], gmax[:, :], None,
                    op0=mybir.AluOpType.mult,
                )
                idx8 = mpool.tile([128, 8], mybir.dt.uint32, tag="idx8")
                nc.vector.max_index(
                    out=idx8[:, :], in_max=gmax8[:, :], in_values=sneg[:, :]
                )
                idxf = mpool.tile([128, 1], f32, tag="idxf")
                nc.vector.tensor_copy(idxf[:, :], idx8[:, 0:1])
                nc.sync.dma_start(out=o_v[it, :], in_=gmax[:, 0])
                nc.sync.dma_start(out=o_i[it, :], in_=idxf[:, 0])
    nc.compile()
    return nc


def _get_nc():
    if "nc" not in _CACHE:
        _CACHE["nc"] = _build_nc()
    return _CACHE["nc"]


def _prep_inputs(desc1, desc2):
    d1 = np.asarray(desc1, dtype=np.float32)
    d2 = np.asarray(desc2, dtype=np.float32)
    q = (d2.astype(np.float64) ** 2).sum(axis=1).astype(np.float32)
    d2t = np.ascontiguousarray(d2.T)
    qnb = np.ascontiguousarray(np.broadcast_to(-q[None, :], (128, B2)))
    in_maps = []
    for c in range(NCORES):
        slab = d1[c * M : (c + 1) * M]
        d1t = np.ascontiguousarray((-2.0 * slab).T)
        in_maps.append({"d1t": d1t, "d2t": d2t, "qnb": qnb})
    return in_maps


def _install_ntff_shim():
    """Register antenv.axon_hooks (missing from this image) so
    run_bass_kernel_spmd(trace=True) can drive NTFF profiling via
    libaxon_pjrt.so's C ABI. Also neuter the cloud artifact upload."""
    import contextlib
    import ctypes
    import sys
    import types

    import concourse.bass_utils as bu

    bu.upload_artifacts = lambda tmpdir: "local://" + str(tmpdir)
    if "antenv.axon_hooks" in sys.modules:
        return
    so_path = "/opt/axon/libaxon_pjrt.so"
    lib = ctypes.CDLL(so_path)
    if not hasattr(lib, "axon_start_nrt_profile"):
        hook = None
    else:
        lib.axon_start_nrt_profile.argtypes = [
            ctypes.POINTER(ctypes.c_int64),
            ctypes.c_size_t,
        ]
        lib.axon_start_nrt_profile.restype = ctypes.c_int64
        lib.axon_stop_nrt_profile.argtypes = [ctypes.c_char_p]
        lib.axon_stop_nrt_profile.restype = ctypes.c_int64

        @contextlib.contextmanager
        def hook(output_dir, device_ids):
            import jax

            jax.devices()
            if device_ids:
                ids = (ctypes.c_int64 * len(device_ids))(*device_ids)
                rc = lib.axon_start_nrt_profile(ids, len(device_ids))
            else:
                rc = lib.axon_start_nrt_profile(None, 0)
            if rc != 0:
                raise RuntimeError(f"axon_start_nrt_profile rc={rc}")
            try:
                yield
            finally:
                n = lib.axon_stop_nrt_profile(str(output_dir).encode())
                print(f"profile: {n} file(s) written to {output_dir}")

    mod = types.ModuleType("antenv.axon_hooks")
    mod.get_axon_ntff_profile_hook = lambda: hook
    mod.set_axon_ntff_profile_hook = lambda h: None
    import antenv

    sys.modules["antenv.axon_hooks"] = mod
    antenv.axon_hooks = mod


def kernel(desc1, desc2, _trace=False, _tmpdir=None):
    from concourse.bass_utils import run_bass_kernel_spmd

    if _trace:
        _install_ntff_shim()

    d1 = np.asarray(desc1, dtype=np.float32)
    nc = _get_nc()
    in_maps = _prep_inputs(d1, desc2)
    res = run_bass_kernel_spmd(
        nc,
        in_maps,
        list(range(NCORES)),
        trace=_trace,
        tmpdir=_tmpdir,
    )
    _CACHE["last_results"] = res

    gmax = np.concatenate(
        [res.results[c]["o_v"].reshape(-1) for c in range(NCORES)]
    )
    idxf = np.concatenate(
        [res.results[c]["o_i"].reshape(-1) for c in range(NCORES)]
    )
    d1sq = (d1.astype(np.float64) ** 2).sum(axis=1).astype(np.float32)
    dist2 = np.maximum(d1sq - gmax, 0.0).astype(np.float32)
    match_dists = np.sqrt(dist2).astype(np.float32)[:, None]
    idxs2 = idxf.astype(np.int32)
    idxs1 = np.arange(B1, dtype=np.int32)
    matches_idxs = np.stack([idxs1, idxs2], axis=1)
    return match_dists, matches_idxs
